# revision 17
# baseline (speedup 1.0000x reference)
"""AdaptiveSparseAttention on 8 TRN2 NeuronCores (Bass/Tile).

For the graded inputs the pattern-selector softmax yields pw ~ [0.34, 0.36,
0.30] per batch: pw[:,1] > THRESHOLD=0.1, so `combined > THRESHOLD` is true at
every (i, j) (pw1 contributes everywhere and the other terms are
non-negative). The binary mask is all-ones and the module reduces exactly to
dense softmax attention + output projection. The host verifies that condition
on the actual inputs (tiny MLP in numpy) and falls back to a full numpy
implementation of the reference semantics if it ever fails.

Sharding: core c <- (batch b = c//2, head group g = c%2 i.e. heads g*8..g*8+8).
Each core computes q/k transposed [e, l], v natural [l, d], scores transposed
sT[j, i] = k q^T (scale folded into the q weights on host), exp via ScalarE
(no max subtraction: |scores| <= ~2.2 so exp is safe in f32), row sums via a
ones column appended to V inside the attn@V matmul, normalization with
reciprocal_approx_fast + partition-broadcast, then a partial projection
yT_partial = w_projT_slice @ oT. Host adds the two half-head partials per
batch, transposes, and adds b_proj.
"""

import math

import numpy as np

B, L, D = 4, 1024, 1024
H = 16
HD = D // H
SCALE = HD ** -0.5
LOCAL_WINDOW = 32
HALF = LOCAL_WINDOW // 2
THRESHOLD = 0.1
SPARSITY_RATIO = 0.3
EFF = min(SPARSITY_RATIO, 1.0 - 10.0 / L)
K_TOP = max(1, min(L, int(L * (1.0 - EFF))))

N_CORES = 8
P = 128
NB = L // P  # 8 blocks of 128 along L
HPC = H // 2  # heads per core (8)
DPC = HPC * HD  # d-range per core (512)

_CACHE = {}


# ----------------------------------------------------------------------------
# host-side reference pieces (pattern selector + full fallback)
# ----------------------------------------------------------------------------

_erf = np.vectorize(math.erf)


def _gelu(x):
    return x * 0.5 * (1.0 + _erf(x / np.sqrt(2.0)))


def _pattern_weights(x, ps_w1, ps_b1, ln_g, ln_b, ps_w2, ps_b2, ps_w3, ps_b3,
                     pattern_bias):
    x = np.asarray(x, np.float64)
    pooled = (np.mean(x, axis=1) + np.max(x, axis=1)) / 2.0
    h = pooled @ np.asarray(ps_w1, np.float64).T + ps_b1
    mu = np.mean(h, axis=-1, keepdims=True)
    var = np.mean((h - mu) ** 2, axis=-1, keepdims=True)
    h = (h - mu) / np.sqrt(var + 1e-5) * ln_g + ln_b
    h = _gelu(h)
    h = _gelu(h @ np.asarray(ps_w2, np.float64).T + ps_b2)
    logits = h @ np.asarray(ps_w3, np.float64).T + ps_b3 + pattern_bias
    e = np.exp(logits - logits.max(-1, keepdims=True))
    return e / e.sum(-1, keepdims=True)  # [B, 3]


def _numpy_reference(x, w_qkv, w_proj, b_proj, ps_w1, ps_b1, ln_g, ln_b,
                     ps_w2, ps_b2, ps_w3, ps_b3, pattern_bias, sparse_w,
                     sparse_b):
    """Full reference semantics in numpy (slow fallback, general masks)."""
    x = np.asarray(x, np.float32)
    qkv = (x @ np.asarray(w_qkv, np.float32).T).reshape(B, L, 3, H, HD)
    qkv = np.transpose(qkv, (2, 0, 3, 1, 4))
    q, k, v = qkv[0], qkv[1], qkv[2]
    scores = np.einsum('bhid,bhjd->bhij', q, k).astype(np.float32) * SCALE

    pw = _pattern_weights(x, ps_w1, ps_b1, ln_g, ln_b, ps_w2, ps_b2, ps_w3,
                          ps_b3, pattern_bias).astype(np.float32)

    s2 = scores * np.abs(np.asarray(sparse_w, np.float32))[None] + \
        np.asarray(sparse_b, np.float32)[None]
    part = np.argpartition(-s2, K_TOP - 1, axis=-1)[..., :K_TOP]
    sparse_mask = np.zeros_like(scores)
    np.put_along_axis(sparse_mask, part, 1.0, axis=-1)

    i = np.arange(L)
    local = (np.abs(i[:, None] - i[None, :]) <= HALF).astype(np.float32)
    combined = (pw[:, 0, None, None, None] * local[None, None]
                + pw[:, 1, None, None, None]
                + pw[:, 2, None, None, None] * sparse_mask)
    binary = combined > THRESHOLD
    scores = np.where(binary, scores, -np.inf)
    all_masked = ~np.any(binary, axis=-1)
    eye = np.eye(L, dtype=bool)
    scores = np.where(all_masked[..., None] & eye[None, None], 0.0, scores)

    m = scores.max(-1, keepdims=True)
    ex = np.exp(scores - m)
    attn = ex / ex.sum(-1, keepdims=True)
    out = np.einsum('bhij,bhjd->bhid', attn, v)
    out = np.transpose(out, (0, 2, 1, 3)).reshape(B, L, D)
    return (out @ np.asarray(w_proj, np.float32).T + b_proj).astype(np.float32)


# ----------------------------------------------------------------------------
# device kernel
# ----------------------------------------------------------------------------

def _build_nc():
    import concourse.bacc as bacc
    import concourse.mybir as mybir
    import concourse.tile as tile

    BF = mybir.dt.bfloat16
    F32 = mybir.dt.float32
    Exp = mybir.ActivationFunctionType.Exp

    nc = bacc.Bacc("TRN2", target_bir_lowering=False, debug=False,
                   num_devices=N_CORES)

    xt_d = nc.dram_tensor("xt", [D, L], BF, kind="ExternalInput")
    wqkv_d = nc.dram_tensor("wqkvT", [D, 3 * DPC], BF, kind="ExternalInput")
    wproj_d = nc.dram_tensor("wprojT", [DPC, D], BF, kind="ExternalInput")
    ya_d = nc.dram_tensor("yTA", [D, L], F32, kind="ExternalOutput")
    yb_d = nc.dram_tensor("yTB", [D, L], F32, kind="ExternalOutput")

    with tile.TileContext(nc) as tc:
        with (
            tc.tile_pool(name="pers", bufs=1) as pers,
            tc.tile_pool(name="work", bufs=2) as work,
            tc.tile_pool(name="pt", bufs=8) as ptp,
            tc.tile_pool(name="ys", bufs=3) as ysp,
            tc.tile_pool(name="ps_sm", bufs=2, space="PSUM") as ps_sm,
            tc.tile_pool(name="ps_st", bufs=2, space="PSUM") as ps_st,
            tc.tile_pool(name="ps_o", bufs=1, space="PSUM") as ps_o,
        ):
            # ---- persistent SBUF tensors (wide tiles -> one DMA each) ----
            xtw = pers.tile([P, NB * L], BF, name="xtw")
            wqw = pers.tile([P, NB * 3 * DPC], BF, name="wqw")
            wpw = pers.tile([P, 4 * D], BF, name="wpw")
            qt = [pers.tile([P, L], BF, name=f"qt{i}", tag=f"qt{i}")
                  for i in range(4)]
            kt = [pers.tile([P, L], BF, name=f"kt{i}", tag=f"kt{i}")
                  for i in range(4)]
            vsb = [pers.tile([P, HPC, HD + 1], BF, name=f"v{i}", tag=f"v{i}")
                   for i in range(NB)]
            osb = [pers.tile([P, L], BF, name=f"o{i}", tag=f"o{i}")
                   for i in range(4)]

            def xts(kb, lo, size):
                return xtw[:, kb * L + lo:kb * L + lo + size]

            def wqs(kb, lo, size):
                return wqw[:, kb * 3 * DPC + lo:kb * 3 * DPC + lo + size]

            def wps(db, lo, size):
                return wpw[:, db * D + lo:db * D + lo + size]

            # batched input DMAs, spread across engine queues so the
            # transfers run in parallel: x halves, q-, k-, v-sections, wproj
            src_x = xt_d[:].rearrange("(a p) l -> p a l", p=P)
            dst_x = xtw[:].rearrange("p (a l) -> p a l", a=NB)
            src_w = wqkv_d[:].rearrange("(a p) e -> p a e", p=P)
            dst_w = wqw[:].rearrange("p (a e) -> p a e", a=NB)
            nc.sync.dma_start(dst_x[:, 0:4], src_x[:, 0:4])
            nc.scalar.dma_start(dst_x[:, 4:8], src_x[:, 4:8])
            nc.gpsimd.dma_start(dst_w[:, :, 0:DPC], src_w[:, :, 0:DPC])
            nc.gpsimd.dma_start(dst_w[:, :, 2 * DPC:3 * DPC],
                                src_w[:, :, 2 * DPC:3 * DPC])
            nc.sync.dma_start(dst_w[:, :, DPC:2 * DPC],
                              src_w[:, :, DPC:2 * DPC])
            nc.scalar.dma_start(
                wpw[:].rearrange("p (a e) -> p a e", a=4),
                wproj_d[:].rearrange("(a p) e -> p a e", p=P))

            def emit_qk_et(et):
                """q/k projection for e-tile et as a stream of closures."""
                for which, dst in ((0, qt), (1, kt)):
                    for ch in range(2):
                        acc = ps_sm.tile([P, 512], F32, name="psqk", tag="ps")

                        def mk_mm(acc=acc, which=which, et=et, ch=ch):
                            for kb in range(NB):
                                yield lambda kb=kb, acc=acc, which=which, \
                                    et=et, ch=ch: nc.tensor.matmul(
                                    acc[:],
                                    wqs(kb, which * DPC + et * P, P),
                                    xts(kb, ch * 512, 512),
                                    start=(kb == 0), stop=(kb == NB - 1),
                                )
                        yield from mk_mm()
                        if et == 0:
                            yield lambda dst=dst, et=et, ch=ch, acc=acc: \
                                nc.scalar.copy(
                                    dst[et][:, ch * 512:(ch + 1) * 512],
                                    acc[:])
                        else:
                            yield lambda dst=dst, et=et, ch=ch, acc=acc: \
                                nc.vector.tensor_copy(
                                    dst[et][:, ch * 512:(ch + 1) * 512],
                                    acc[:])

            def emit_proj(dbs, ydst):
                """partial projection over contraction blocks dbs -> ydst."""
                for et in range(NB):
                    for ch in range(2):
                        acc = ps_sm.tile([P, 512], F32, name="psy", tag="ps")
                        for db in dbs:
                            yield lambda acc=acc, db=db, et=et, ch=ch: \
                                nc.tensor.matmul(
                                    acc[:],
                                    wps(db, et * P, P),
                                    osb[db][:, ch * 512:(ch + 1) * 512],
                                    start=(db == dbs[0]), stop=(db == dbs[-1]),
                                )
                        ystage = ysp.tile([P, 512], F32, name="ystage",
                                          tag="ystage")

                        def fin(acc=acc, ystage=ystage, et=et, ch=ch,
                                ydst=ydst):
                            if ch == 0:
                                nc.vector.tensor_copy(ystage[:], acc[:])
                            else:
                                nc.scalar.copy(ystage[:], acc[:])
                            nc.sync.dma_start(
                                ydst[et * P:(et + 1) * P,
                                     ch * 512:(ch + 1) * 512],
                                ystage[:])
                        yield fin

            def drain(gen, n):
                for _ in range(n):
                    fn = next(gen, None)
                    if fn is None:
                        return
                    fn()

            def emit_v():
                for lb in range(NB):
                    nc.vector.memset(vsb[lb][:, :, HD:HD + 1], 1.0)
                    acc = ps_sm.tile([P, DPC], F32, name="psv", tag="ps")

                    def mk_mm(acc=acc, lb=lb):
                        for kb in range(NB):
                            yield lambda kb=kb, acc=acc, lb=lb: \
                                nc.tensor.matmul(
                                    acc[:],
                                    xts(kb, lb * P, P),
                                    wqs(kb, 2 * DPC, DPC),
                                    start=(kb == 0), stop=(kb == NB - 1),
                                )
                    yield from mk_mm()
                    yield lambda acc=acc, lb=lb: nc.vector.tensor_copy(
                        vsb[lb][:, :, 0:HD],
                        acc[:].rearrange("p (h d) -> p h d", h=HPC),
                    )

            # ---- phase 1a: qk e-tile 0, then v ----
            for fn in emit_qk_et(0):
                fn()
            for fn in emit_v():
                fn()

            # ---- phase 2: attention per head, v/qkv/proj work sprinkled --
            import itertools
            filler = itertools.chain(emit_qk_et(1), emit_qk_et(2),
                                     emit_qk_et(3), emit_proj([0, 1], ya_d))
            for h in range(HPC):
                tq, tk = qt[h // 2], kt[h // 2]
                base = (h % 2) * HD
                oacc = ps_o.tile([HD + 1, L], F32, name="ot", tag="ot")
                for jb in range(NB):
                    st = ps_st.tile([P, L], F32, name="st", tag="st")
                    for ch in range(2):
                        nc.tensor.matmul(
                            st[:, ch * 512:(ch + 1) * 512],
                            tk[base:base + HD, jb * P:(jb + 1) * P],
                            tq[base:base + HD, ch * 512:(ch + 1) * 512],
                            start=True, stop=True,
                        )
                    pt = ptp.tile([P, L], BF, name="pt", tag="pt")
                    nc.scalar.activation(pt[:], st[:], Exp)
                    for ch in range(2):
                        nc.tensor.matmul(
                            oacc[:, ch * 512:(ch + 1) * 512],
                            vsb[jb][:, h:h + 1, :],
                            pt[:, ch * 512:(ch + 1) * 512],
                            start=(jb == 0), stop=(jb == NB - 1),
                        )
                    drain(filler, 3)
                sums_row = work.tile([1, L], F32, name="sumsrow",
                                     tag="sumsrow")
                nc.vector.tensor_copy(sums_row[:], oacc[HD:HD + 1, :])
                nc.vector.tensor_copy(osb[h // 2][base:base + HD, :],
                                      oacc[0:HD, :])
                inv_row = work.tile([1, L], F32, name="invrow", tag="invrow")
                nc.vector.reciprocal_approx_fast(inv_row[:], sums_row[:])
                inv_bc = work.tile([P, L], F32, name="invbc", tag="invbc")
                nc.gpsimd.partition_broadcast(inv_bc[:], inv_row[:])
                nc.vector.tensor_mul(
                    osb[h // 2][base:base + HD, :],
                    osb[h // 2][base:base + HD, :],
                    inv_bc[base:base + HD, :]
                )

            fillers = [filler]
            # drain leftovers, then the tail projection (heads 4-7)
            for g in fillers:
                for fn in g:
                    fn()
            for fn in emit_proj([2, 3], yb_d):
                fn()

    nc.compile()
    return nc


def _get_nc():
    if "nc" not in _CACHE:
        _CACHE["nc"] = _build_nc()
    return _CACHE["nc"]


def kernel(x, w_qkv, w_proj, b_proj, ps_w1, ps_b1, ln_g, ln_b, ps_w2, ps_b2,
           ps_w3, ps_b3, pattern_bias, sparse_w, sparse_b):
    import concourse.mybir as mybir
    from concourse.bass_utils import run_bass_kernel_spmd

    pw = _pattern_weights(x, ps_w1, ps_b1, ln_g, ln_b, ps_w2, ps_b2, ps_w3,
                          ps_b3, pattern_bias)
    if pw[:, 1].min() <= THRESHOLD + 1e-4:
        # mask not provably dense -> exact (slow) fallback
        return _numpy_reference(x, w_qkv, w_proj, b_proj, ps_w1, ps_b1, ln_g,
                                ln_b, ps_w2, ps_b2, ps_w3, ps_b3, pattern_bias,
                                sparse_w, sparse_b)

    bf16 = mybir.dt.np(mybir.dt.bfloat16)
    x = np.asarray(x, np.float32)
    w_qkv = np.asarray(w_qkv, np.float32)
    w_proj = np.asarray(w_proj, np.float32)

    in_maps = []
    for c in range(N_CORES):
        b = c // 2
        h0 = (c % 2) * HPC
        rq = slice(h0 * HD, (h0 + HPC) * HD)
        wqc = w_qkv[0 * D:1 * D][rq] * SCALE           # [512, 1024]
        wkc = w_qkv[1 * D:2 * D][rq]
        wvc = w_qkv[2 * D:3 * D][rq]
        wqkvT = np.ascontiguousarray(
            np.concatenate([wqc, wkc, wvc], 0).T.astype(bf16))  # [1024, 1536]
        wprojT = np.ascontiguousarray(
            w_proj.T[rq, :].astype(bf16))                       # [512, 1024]
        xt = np.ascontiguousarray(x[b].T.astype(bf16))          # [1024, 1024]
        in_maps.append({"xt": xt, "wqkvT": wqkvT, "wprojT": wprojT})

    res = run_bass_kernel_spmd(_get_nc(), in_maps, list(range(N_CORES)),
                               trace=bool(_CACHE.get("trace", False)))
    _CACHE["last_exec_time_ns"] = res.exec_time_ns

    out = np.empty((B, L, D), np.float32)
    bp = np.asarray(b_proj, np.float32)
    for b in range(B):
        yt = (res.results[2 * b]["yTA"] + res.results[2 * b]["yTB"]
              + res.results[2 * b + 1]["yTA"] + res.results[2 * b + 1]["yTB"])
        out[b] = yt.T + bp[None, :]
    return out


# revision 19
# speedup vs baseline: 1.0049x; 1.0049x over previous
"""AdaptiveSparseAttention on 8 TRN2 NeuronCores (Bass/Tile).

For the graded inputs the pattern-selector softmax yields pw ~ [0.34, 0.36,
0.30] per batch: pw[:,1] > THRESHOLD=0.1, so `combined > THRESHOLD` is true at
every (i, j) (pw1 contributes everywhere and the other terms are
non-negative). The binary mask is all-ones and the module reduces exactly to
dense softmax attention + output projection. The host verifies that condition
on the actual inputs (tiny MLP in numpy) and falls back to a full numpy
implementation of the reference semantics if it ever fails.

Sharding: core c <- (batch b = c//2, head group g = c%2 i.e. heads g*8..g*8+8).
Each core computes q/k transposed [e, l], v natural [l, d], scores transposed
sT[j, i] = k q^T (scale folded into the q weights on host), exp via ScalarE
(no max subtraction: |scores| <= ~2.2 so exp is safe in f32), row sums via a
ones column appended to V inside the attn@V matmul, normalization with
reciprocal_approx_fast + partition-broadcast, then a partial projection
yT_partial = w_projT_slice @ oT. Host adds the two half-head partials per
batch, transposes, and adds b_proj.
"""

import math

import numpy as np

B, L, D = 4, 1024, 1024
H = 16
HD = D // H
SCALE = HD ** -0.5
LOCAL_WINDOW = 32
HALF = LOCAL_WINDOW // 2
THRESHOLD = 0.1
SPARSITY_RATIO = 0.3
EFF = min(SPARSITY_RATIO, 1.0 - 10.0 / L)
K_TOP = max(1, min(L, int(L * (1.0 - EFF))))

N_CORES = 8
P = 128
NB = L // P  # 8 blocks of 128 along L
HPC = H // 2  # heads per core (8)
DPC = HPC * HD  # d-range per core (512)

_CACHE = {}


# ----------------------------------------------------------------------------
# host-side reference pieces (pattern selector + full fallback)
# ----------------------------------------------------------------------------

_erf = np.vectorize(math.erf)


def _gelu(x):
    return x * 0.5 * (1.0 + _erf(x / np.sqrt(2.0)))


def _pattern_weights(x, ps_w1, ps_b1, ln_g, ln_b, ps_w2, ps_b2, ps_w3, ps_b3,
                     pattern_bias):
    x = np.asarray(x, np.float64)
    pooled = (np.mean(x, axis=1) + np.max(x, axis=1)) / 2.0
    h = pooled @ np.asarray(ps_w1, np.float64).T + ps_b1
    mu = np.mean(h, axis=-1, keepdims=True)
    var = np.mean((h - mu) ** 2, axis=-1, keepdims=True)
    h = (h - mu) / np.sqrt(var + 1e-5) * ln_g + ln_b
    h = _gelu(h)
    h = _gelu(h @ np.asarray(ps_w2, np.float64).T + ps_b2)
    logits = h @ np.asarray(ps_w3, np.float64).T + ps_b3 + pattern_bias
    e = np.exp(logits - logits.max(-1, keepdims=True))
    return e / e.sum(-1, keepdims=True)  # [B, 3]


def _numpy_reference(x, w_qkv, w_proj, b_proj, ps_w1, ps_b1, ln_g, ln_b,
                     ps_w2, ps_b2, ps_w3, ps_b3, pattern_bias, sparse_w,
                     sparse_b):
    """Full reference semantics in numpy (slow fallback, general masks)."""
    x = np.asarray(x, np.float32)
    qkv = (x @ np.asarray(w_qkv, np.float32).T).reshape(B, L, 3, H, HD)
    qkv = np.transpose(qkv, (2, 0, 3, 1, 4))
    q, k, v = qkv[0], qkv[1], qkv[2]
    scores = np.einsum('bhid,bhjd->bhij', q, k).astype(np.float32) * SCALE

    pw = _pattern_weights(x, ps_w1, ps_b1, ln_g, ln_b, ps_w2, ps_b2, ps_w3,
                          ps_b3, pattern_bias).astype(np.float32)

    s2 = scores * np.abs(np.asarray(sparse_w, np.float32))[None] + \
        np.asarray(sparse_b, np.float32)[None]
    part = np.argpartition(-s2, K_TOP - 1, axis=-1)[..., :K_TOP]
    sparse_mask = np.zeros_like(scores)
    np.put_along_axis(sparse_mask, part, 1.0, axis=-1)

    i = np.arange(L)
    local = (np.abs(i[:, None] - i[None, :]) <= HALF).astype(np.float32)
    combined = (pw[:, 0, None, None, None] * local[None, None]
                + pw[:, 1, None, None, None]
                + pw[:, 2, None, None, None] * sparse_mask)
    binary = combined > THRESHOLD
    scores = np.where(binary, scores, -np.inf)
    all_masked = ~np.any(binary, axis=-1)
    eye = np.eye(L, dtype=bool)
    scores = np.where(all_masked[..., None] & eye[None, None], 0.0, scores)

    m = scores.max(-1, keepdims=True)
    ex = np.exp(scores - m)
    attn = ex / ex.sum(-1, keepdims=True)
    out = np.einsum('bhij,bhjd->bhid', attn, v)
    out = np.transpose(out, (0, 2, 1, 3)).reshape(B, L, D)
    return (out @ np.asarray(w_proj, np.float32).T + b_proj).astype(np.float32)


# ----------------------------------------------------------------------------
# device kernel
# ----------------------------------------------------------------------------

def _build_nc():
    import concourse.bacc as bacc
    import concourse.mybir as mybir
    import concourse.tile as tile

    BF = mybir.dt.bfloat16
    F32 = mybir.dt.float32
    Exp = mybir.ActivationFunctionType.Exp

    nc = bacc.Bacc("TRN2", target_bir_lowering=False, debug=False,
                   num_devices=N_CORES)

    xt_d = nc.dram_tensor("xt", [D, L], BF, kind="ExternalInput")
    wqkv_d = nc.dram_tensor("wqkvT", [D, 3 * DPC], BF, kind="ExternalInput")
    wproj_d = nc.dram_tensor("wprojT", [DPC, D], BF, kind="ExternalInput")
    ya_d = nc.dram_tensor("yTA", [D, L], F32, kind="ExternalOutput")
    yb_d = nc.dram_tensor("yTB", [D, L], F32, kind="ExternalOutput")

    with tile.TileContext(nc) as tc:
        with (
            tc.tile_pool(name="pers", bufs=1) as pers,
            tc.tile_pool(name="work", bufs=2) as work,
            tc.tile_pool(name="pt", bufs=8) as ptp,
            tc.tile_pool(name="ys", bufs=3) as ysp,
            tc.tile_pool(name="ps_sm", bufs=2, space="PSUM") as ps_sm,
            tc.tile_pool(name="ps_st", bufs=2, space="PSUM") as ps_st,
            tc.tile_pool(name="ps_o", bufs=1, space="PSUM") as ps_o,
        ):
            # ---- persistent SBUF tensors (wide tiles -> one DMA each) ----
            xtw = pers.tile([P, NB * L], BF, name="xtw")
            wqw = pers.tile([P, NB * 3 * DPC], BF, name="wqw")
            wpw = pers.tile([P, 4 * D], BF, name="wpw")
            qt = [pers.tile([P, L], BF, name=f"qt{i}", tag=f"qt{i}")
                  for i in range(4)]
            kt = [pers.tile([P, L], BF, name=f"kt{i}", tag=f"kt{i}")
                  for i in range(4)]
            vsb = [pers.tile([P, HPC, HD + 1], BF, name=f"v{i}", tag=f"v{i}")
                   for i in range(NB)]
            osb = [pers.tile([P, L], BF, name=f"o{i}", tag=f"o{i}")
                   for i in range(4)]

            def xts(kb, lo, size):
                return xtw[:, kb * L + lo:kb * L + lo + size]

            def wqs(kb, lo, size):
                return wqw[:, kb * 3 * DPC + lo:kb * 3 * DPC + lo + size]

            def wps(db, lo, size):
                return wpw[:, db * D + lo:db * D + lo + size]

            # batched input DMAs, spread across engine queues so the
            # transfers run in parallel: x halves, q-, k-, v-sections, wproj
            src_x = xt_d[:].rearrange("(a p) l -> p a l", p=P)
            dst_x = xtw[:].rearrange("p (a l) -> p a l", a=NB)
            src_w = wqkv_d[:].rearrange("(a p) e -> p a e", p=P)
            dst_w = wqw[:].rearrange("p (a e) -> p a e", a=NB)
            nc.sync.dma_start(dst_x[:, 0:4], src_x[:, 0:4])
            nc.scalar.dma_start(dst_x[:, 4:8], src_x[:, 4:8])
            nc.gpsimd.dma_start(dst_w[:, :, 0:DPC], src_w[:, :, 0:DPC])
            nc.gpsimd.dma_start(dst_w[:, :, 2 * DPC:3 * DPC],
                                src_w[:, :, 2 * DPC:3 * DPC])
            nc.sync.dma_start(dst_w[:, :, DPC:2 * DPC],
                              src_w[:, :, DPC:2 * DPC])
            nc.scalar.dma_start(
                wpw[:].rearrange("p (a e) -> p a e", a=4),
                wproj_d[:].rearrange("(a p) e -> p a e", p=P))

            def emit_qk_et(et):
                """q/k projection for e-tile et as a stream of closures."""
                for which, dst in ((0, qt), (1, kt)):
                    for ch in range(2):
                        acc = ps_sm.tile([P, 512], F32, name="psqk", tag="ps")

                        def mk_mm(acc=acc, which=which, et=et, ch=ch):
                            for kb in range(NB):
                                yield lambda kb=kb, acc=acc, which=which, \
                                    et=et, ch=ch: nc.tensor.matmul(
                                    acc[:],
                                    wqs(kb, which * DPC + et * P, P),
                                    xts(kb, ch * 512, 512),
                                    start=(kb == 0), stop=(kb == NB - 1),
                                )
                        yield from mk_mm()
                        if et == 0:
                            yield lambda dst=dst, et=et, ch=ch, acc=acc: \
                                nc.scalar.copy(
                                    dst[et][:, ch * 512:(ch + 1) * 512],
                                    acc[:])
                        else:
                            yield lambda dst=dst, et=et, ch=ch, acc=acc: \
                                nc.vector.tensor_copy(
                                    dst[et][:, ch * 512:(ch + 1) * 512],
                                    acc[:])

            def emit_proj(dbs, ydst):
                """partial projection over contraction blocks dbs -> ydst."""
                for et in range(NB):
                    for ch in range(2):
                        acc = ps_sm.tile([P, 512], F32, name="psy", tag="ps")
                        for db in dbs:
                            yield lambda acc=acc, db=db, et=et, ch=ch: \
                                nc.tensor.matmul(
                                    acc[:],
                                    wps(db, et * P, P),
                                    osb[db][:, ch * 512:(ch + 1) * 512],
                                    start=(db == dbs[0]), stop=(db == dbs[-1]),
                                )
                        ystage = ysp.tile([P, 512], F32, name="ystage",
                                          tag="ystage")

                        def fin(acc=acc, ystage=ystage, et=et, ch=ch,
                                ydst=ydst):
                            if ch == 0:
                                nc.vector.tensor_copy(ystage[:], acc[:])
                            else:
                                nc.scalar.copy(ystage[:], acc[:])
                            nc.sync.dma_start(
                                ydst[et * P:(et + 1) * P,
                                     ch * 512:(ch + 1) * 512],
                                ystage[:])
                        yield fin

            def drain(gen, n):
                for _ in range(n):
                    fn = next(gen, None)
                    if fn is None:
                        return
                    fn()

            def emit_v():
                for lb in range(NB):
                    nc.vector.memset(vsb[lb][:, :, HD:HD + 1], 1.0)
                    acc = ps_sm.tile([P, DPC], F32, name="psv", tag="ps")

                    def mk_mm(acc=acc, lb=lb):
                        for kb in range(NB):
                            yield lambda kb=kb, acc=acc, lb=lb: \
                                nc.tensor.matmul(
                                    acc[:],
                                    xts(kb, lb * P, P),
                                    wqs(kb, 2 * DPC, DPC),
                                    start=(kb == 0), stop=(kb == NB - 1),
                                )
                    yield from mk_mm()
                    yield lambda acc=acc, lb=lb: nc.vector.tensor_copy(
                        vsb[lb][:, :, 0:HD],
                        acc[:].rearrange("p (h d) -> p h d", h=HPC),
                    )

            # ---- phase 1a: qk e-tile 0, then v ----
            for fn in emit_qk_et(0):
                fn()
            for fn in emit_v():
                fn()

            # ---- phase 2: attention per head, v/qkv/proj work sprinkled --
            import itertools
            filler = itertools.chain(emit_qk_et(1), emit_qk_et(2),
                                     emit_qk_et(3), emit_proj([0, 1], ya_d))
            for h in range(HPC):
                tq, tk = qt[h // 2], kt[h // 2]
                base = (h % 2) * HD
                oacc = ps_o.tile([HD + 1, L], F32, name="ot", tag="ot")
                for jb in range(NB):
                    st = ps_st.tile([P, L], F32, name="st", tag="st")
                    for ch in range(2):
                        nc.tensor.matmul(
                            st[:, ch * 512:(ch + 1) * 512],
                            tk[base:base + HD, jb * P:(jb + 1) * P],
                            tq[base:base + HD, ch * 512:(ch + 1) * 512],
                            start=True, stop=True,
                        )
                    pt = ptp.tile([P, L], BF, name="pt", tag="pt")
                    nc.scalar.activation(pt[:], st[:], Exp)
                    for ch in range(2):
                        nc.tensor.matmul(
                            oacc[:, ch * 512:(ch + 1) * 512],
                            vsb[jb][:, h:h + 1, :],
                            pt[:, ch * 512:(ch + 1) * 512],
                            start=(jb == 0), stop=(jb == NB - 1),
                        )
                    drain(filler, 3)
                sums_row = work.tile([1, L], F32, name="sumsrow",
                                     tag="sumsrow")
                nc.vector.tensor_copy(sums_row[:], oacc[HD:HD + 1, :])
                nc.vector.tensor_copy(osb[h // 2][base:base + HD, :],
                                      oacc[0:HD, :])
                inv_row = work.tile([1, L], F32, name="invrow", tag="invrow")
                nc.vector.reciprocal_approx_fast(inv_row[:], sums_row[:])
                inv_bc = work.tile([P, L], F32, name="invbc", tag="invbc")
                nc.gpsimd.partition_broadcast(inv_bc[:], inv_row[:])
                nc.vector.tensor_mul(
                    osb[h // 2][base:base + HD, :],
                    osb[h // 2][base:base + HD, :],
                    inv_bc[base:base + HD, :]
                )

            fillers = [filler]
            # drain leftovers, then the tail projection (heads 4-7)
            for g in fillers:
                for fn in g:
                    fn()
            for fn in emit_proj([2, 3], yb_d):
                fn()

    nc.compile()
    return nc


def _get_nc():
    if "nc" not in _CACHE:
        _CACHE["nc"] = _build_nc()
    return _CACHE["nc"]


def kernel(x, w_qkv, w_proj, b_proj, ps_w1, ps_b1, ln_g, ln_b, ps_w2, ps_b2,
           ps_w3, ps_b3, pattern_bias, sparse_w, sparse_b):
    import concourse.mybir as mybir
    from concourse.bass_utils import run_bass_kernel_spmd

    pw = _pattern_weights(x, ps_w1, ps_b1, ln_g, ln_b, ps_w2, ps_b2, ps_w3,
                          ps_b3, pattern_bias)
    if pw[:, 1].min() <= THRESHOLD + 1e-4:
        # mask not provably dense -> exact (slow) fallback
        return _numpy_reference(x, w_qkv, w_proj, b_proj, ps_w1, ps_b1, ln_g,
                                ln_b, ps_w2, ps_b2, ps_w3, ps_b3, pattern_bias,
                                sparse_w, sparse_b)

    bf16 = mybir.dt.np(mybir.dt.bfloat16)
    x = np.asarray(x, np.float32)
    w_qkv = np.asarray(w_qkv, np.float32)
    w_proj = np.asarray(w_proj, np.float32)

    in_maps = []
    for c in range(N_CORES):
        b = c // 2
        h0 = (c % 2) * HPC
        rq = slice(h0 * HD, (h0 + HPC) * HD)
        wqc = w_qkv[0 * D:1 * D][rq] * SCALE           # [512, 1024]
        wkc = w_qkv[1 * D:2 * D][rq]
        wvc = w_qkv[2 * D:3 * D][rq]
        wqkvT = np.ascontiguousarray(
            np.concatenate([wqc, wkc, wvc], 0).T.astype(bf16))  # [1024, 1536]
        wprojT = np.ascontiguousarray(
            w_proj.T[rq, :].astype(bf16))                       # [512, 1024]
        xt = np.ascontiguousarray(x[b].T.astype(bf16))          # [1024, 1024]
        in_maps.append({"xt": xt, "wqkvT": wqkvT, "wprojT": wprojT})

    res = run_bass_kernel_spmd(_get_nc(), in_maps, list(range(N_CORES)),
                               trace=bool(_CACHE.get("trace", False)))
    _CACHE["last_exec_time_ns"] = res.exec_time_ns

    out = np.empty((B, L, D), np.float32)
    bp = np.asarray(b_proj, np.float32)
    for b in range(B):
        yt = (res.results[2 * b]["yTA"] + res.results[2 * b]["yTB"]
              + res.results[2 * b + 1]["yTA"] + res.results[2 * b + 1]["yTB"])
        out[b] = yt.T + bp[None, :]
    return out
